# revision 20
# baseline (speedup 1.0000x reference)
"""nn_GCNConv Trainium2 Bass kernel (8 NeuronCores, SPMD, no collectives).

Computation: out = segment_sum(features[src], dst, N) @ W + b
  features [10000,128] f32, edge_index [2,640000] i64, W [128,256], b [256]

Strategy (dense count-matrix SpMM -> pure streaming GEMM, no SWDGE gather):
  - The segment-sum is  agg = A @ F  where A[d,s] = #edges s->d.  The host
    builds A as a dense fp8 count matrix (counts are tiny ints, exact in
    fp8e4) sharded by dst: core c owns dst nodes [1280c, 1280c+1280).
  - dst is sharded 1250/core (no dst padding); GROUP-MAJOR schedule:
    F ([128,79,128] bf16) lives fully in SBUF; per dst group g
    (512/512/226 columns) the PE accumulates
      aggT[f,dg] += F_chunk[s,f]^T @ A_g_chunk[s,dg]
    over all 79 src chunks into one persistent PSUM bank, then group g's
    stage 2 (per-window PSUM->SBUF copy, aggT^T @ W + b, output DMA) runs
    while group g+1's A is still streaming -- only the last (smallest)
    group's stage 2 is on the tail.  Stage-2 PE matmuls are emitted a few
    chunks into the next group so the DVE copies hide under accumulation
    matmuls (and fill the early wire-starved phase).
  - A streams on the sync HWDGE ring in consumption order (g0,g1,g2); F
    streams on the scalar ring ahead of its per-chunk use, followed by
    w/b and the output stores (keeping the early SDMA round-robin 2-way).
    Slabs ramp small->~0.9MB: the 8 shared DMA-lane sems cap DMAs in
    flight, so tiny slabs choke the ~420 GB/s wire, while coarse slabs
    stall the PE at slab-sem boundaries (+~2us completion receipt each).
    Zero-weight warm-up matmuls bridge the DMA head so the PE HAM clock
    gate reaches 2.4 GHz before real work and never re-throttles.
  - walrus's codegen epilogue clears all 256 semaphores one
    EVENT_SEMAPHORE each (~6.6us, unavoidable); max-sem-num=78 keeps bass
    sems compact (harmless, kept from the attempt to shrink that loop).
  - Output is written p-major ([128,10,256] f16, last window 98 rows
    valid) so stores are big-line DMAs; the host untransposes.  Host does
    only index bookkeeping (histogram/pack); all feature arithmetic runs
    on device.
"""

import sys

import ml_dtypes
import numpy as np

_TRN_REPO = "/opt/trn_rl_repo"
if _TRN_REPO not in sys.path:
    sys.path.insert(0, _TRN_REPO)

import concourse.bass as bass  # noqa: E402
import concourse.mybir as mybir  # noqa: E402
import concourse.tile as tile  # noqa: E402
from concourse import bacc, bass_utils  # noqa: E402

# ---------------------------------------------------------------------------
# The walrus codegen epilogue clears every semaphore in the 256-entry file
# one EVENT_SEMAPHORE per sem (~115 ns each, split across 5 engines) — ~6 us
# of measured kernel time.  The sem file split is governed by max-sem-num
# (walrus owns [0, N), bass kernels own [N, 256)); shrinking N and telling
# bass to allocate in the small range right above it shrinks the clear loop.
# 78 is the documented-safe walrus minimum (the RDH configuration value).
_MAX_SEM_NUM = 78


def _patched_max_sem_num() -> int:
    return _MAX_SEM_NUM


bass.get_walrus_max_sem_num = _patched_max_sem_num

_orig_bir_verify = bass_utils.bir_verify_and_optimise


def _bir_verify_with_sem_cap(*args, **kwargs):
    orig_get_args = bass_utils.get_walrus_args

    def patched_get_args(*a, **k):
        return orig_get_args(*a, **k) + [f"--max-sem-num={_MAX_SEM_NUM}"]

    bass_utils.get_walrus_args = patched_get_args
    try:
        return _orig_bir_verify(*args, **kwargs)
    finally:
        bass_utils.get_walrus_args = orig_get_args


bass_utils.bir_verify_and_optimise = _bir_verify_with_sem_cap
# ---------------------------------------------------------------------------

# ---------------------------------------------------------------------------
# Workaround: this walrus build rejects >1 sync-wait on a CTRL instruction
# ("Too many sync wait commands"). Tile's tail drain attaches a wait for every
# live sem lane to one InstDrain; chunk them across single-wait nops instead.
import re as _re  # noqa: E402

import bass_rust as _bass_rust  # noqa: E402


def _clock_ticks(vc):
    m = _re.search(r"\[([0-9, ]*)\]", repr(vc))
    return [int(x) for x in m.group(1).split(",")] if m.group(1).strip() else []


def _drain_and_barrier(self, tick_clock, wait_clock):
    ticks = _clock_ticks(tick_clock.global_clock)
    nz = [(i, t) for i, t in enumerate(ticks) if t > 0]
    for i, t in nz:
        vc = _bass_rust.VectorClock()
        vc.require_at_least(i, t)
        nop = self.nc.sync.nop(nofuse=True, hint="tail_wait")
        wait_clock.add_sem_waits(nop.ins, tile.ScopedClock({None: vc}))
    self.nc.sync.drain()  # waits already carried by the nops (SP FIFO order)
    self.nc.all_engine_barrier()
    assert self.sems is not None
    popped = self.nc._tile_sem_poison_stack.pop()
    assert popped is self._sem_poison
    self.nc.clear_and_free_semaphores(list(self.sems.allocated().values()))
    self.nc.all_engine_barrier()


tile.TileContext._drain_and_barrier = _drain_and_barrier
# ---------------------------------------------------------------------------

P = 128            # SBUF partitions = window node count = src chunk size
C_IN = 128
C_OUT = 256
N_NODES = 10000
N_CORES = 8
DPC = 1250         # dst nodes per core (exactly 10000/8 -- no dst padding)
NWIN = 10          # output windows per core (last window only 98 rows valid)
NCH = 79           # src chunks (10112 padded src rows / 128)
# Group order 512,512,226: the wide groups keep the PE stream dense early
# (a small-group-first variant starved the PE into HAM re-throttling);
# the small group last also gives the shortest stage-2 tail.
GROUPS = (512, 512, 226)   # dst columns per PSUM accumulation group
GROUP_BOUNDS = ((0, 512), (512, 1024), (1024, 1250))
GROUP_WBASE = (0, 4, 8)
# Slab sizes (src chunks per DMA).  All A slabs ride the sync HWDGE ring in
# consumption order (g0, g1, g2); F rides the scalar ring ahead of its use,
# followed by the output stores.  Slabs are kept >=0.6 MB mid-stream — the
# 8 shared DMA-lane sems cap DMAs in flight, and with small slabs the
# in-flight byte pool can't sustain the ~420 GB/s the wire delivers.
A_SLABS = (
    (4, 6, 8, 10, 12, 13, 13, 13),
    (13, 13, 13, 13, 13, 14),
    (20, 20, 20, 19),
)
F_SLABS = (4, 6, 8, 10, 12, 13, 13, 13)
N_WARM = 17        # dummy matmuls bridging the DMA head (HAM pre-warm)
S2_DEFER = 6      # emit group g's stage-2 this many chunks into group g+1


def _build_kernel():
    nc = bacc.Bacc("TRN2")
    dt = mybir.dt

    a_d = [
        nc.dram_tensor(f"a{gi}", [P, NCH, ng], dt.float8e4, kind="ExternalInput")
        for gi, ng in enumerate(GROUPS)
    ]
    f_d = nc.dram_tensor("f", [P, NCH, C_IN], dt.bfloat16, kind="ExternalInput")
    w_d = nc.dram_tensor("w", [C_IN, C_OUT], dt.bfloat16, kind="ExternalInput")
    bb_d = nc.dram_tensor("bb", [P, C_OUT], dt.float32, kind="ExternalInput")
    out_d = nc.dram_tensor("out", [P, NWIN, C_OUT], dt.float16, kind="ExternalOutput")

    with tile.TileContext(nc) as tc:
        with (
            tc.tile_pool(name="consts", bufs=1) as cpool,
            tc.tile_pool(name="a", bufs=1) as apool,
            tc.tile_pool(name="agg", bufs=1) as aggpool,
            tc.tile_pool(name="o", bufs=2) as opool,
            tc.tile_pool(name="psa", bufs=1, space="PSUM") as psa,
            tc.tile_pool(name="pso", bufs=1, space="PSUM") as pso,
        ):
            # HAM pre-warm: dummy matmuls on zeroed scratch keep the PE busy
            # during the DMA head so the real stream hits 2.4 GHz sooner.
            warm_w = cpool.tile([P, C_IN], dt.bfloat16)
            warm_x = cpool.tile([P, 256], dt.float8e4)
            warm_p = psa.tile([P, 256], dt.float32, tag="warm", bufs=1)
            nc.gpsimd.memset(warm_w[:], 0.0)
            nc.gpsimd.memset(warm_x[:], 0.0)
            for _ in range(N_WARM):
                nc.tensor.matmul(warm_p[:], lhsT=warm_w[:], rhs=warm_x[:],
                                 start=True, stop=True)

            w_s = cpool.tile([C_IN, C_OUT], dt.bfloat16)
            bb_s = cpool.tile([P, C_OUT], dt.float32)

            # F fully resident + per-group A tiles.  All input slabs are
            # striped across the sync/scalar HWDGE rings in consumption
            # order (region tracking gates each chunk's MM on the slab that
            # carries it).
            f_s = cpool.tile([P, NCH, C_IN], dt.bfloat16)
            a_ss = [
                apool.tile([P, NCH, ng], dt.float8e4, tag=f"g{gi}",
                           name=f"a_s{gi}")
                for gi, ng in enumerate(GROUPS)
            ]

            def slab(ring, dst, src, kb0, kbn):
                ring.dma_start(
                    out=dst[:, kb0 : kb0 + kbn, :], in_=src[:, kb0 : kb0 + kbn, :]
                )

            kb0 = 0
            for kbn in F_SLABS:
                slab(nc.scalar, f_s, f_d, kb0, kbn)
                kb0 += kbn
            # w/b ride the scalar ring AFTER F: issuing them early would make
            # the SDMA round-robin 3-way during the critical F+g0 phase
            nc.scalar.dma_start(out=w_s[:], in_=w_d[:])
            nc.scalar.dma_start(out=bb_s[:], in_=bb_d[:])
            for gi, slabs in enumerate(A_SLABS):
                kb0 = 0
                for kbn in slabs:
                    slab(nc.sync, a_ss[gi], a_d[gi], kb0, kbn)
                    kb0 += kbn

            # persistent PSUM accumulator per group (one bank each)
            aggps = [
                psa.tile([P, ng], dt.float32, tag=f"aggp{gi}", name=f"aggp{gi}")
                for gi, ng in enumerate(GROUPS)
            ]

            def make_stage2(gi, wbase):
                ng = GROUPS[gi]
                nw = (ng + P - 1) // P
                agg_s = aggpool.tile([P, ng], dt.bfloat16, tag=f"agg{gi}",
                                     name=f"agg_s{gi}")
                # per-window copies: window w's stage-2 matmul can start as
                # soon as its slice is copied (shorter tail on the last group)
                for wl in range(nw):
                    w0, w1 = wl * P, min((wl + 1) * P, ng)
                    nc.vector.tensor_copy(agg_s[:, w0:w1], aggps[gi][:, w0:w1])

                def emit_pe():
                    outp = pso.tile([P, nw, C_OUT], dt.float32, tag="op",
                                    padded_shape=[P, 4, C_OUT], name=f"op{gi}")
                    for wl in range(nw):
                        w0, w1 = wl * P, min((wl + 1) * P, ng)
                        nc.tensor.matmul(
                            outp[0 : w1 - w0, wl, :],
                            lhsT=agg_s[:, w0:w1],
                            rhs=w_s[:],
                            start=True,
                            stop=True,
                        )
                    out_t = opool.tile([P, nw, C_OUT], dt.float16, tag="o",
                                       padded_shape=[P, 4, C_OUT], name=f"out_g{gi}")
                    for wl in range(nw):
                        w0, w1 = wl * P, min((wl + 1) * P, ng)
                        nc.vector.tensor_add(out_t[0 : w1 - w0, wl, :],
                                             outp[0 : w1 - w0, wl, :],
                                             bb_s[0 : w1 - w0, :])
                    nc.scalar.dma_start(
                        out=out_d[:, wbase : wbase + nw, :], in_=out_t[:]
                    )

                return emit_pe

            # stage 1 accumulation, group-major; group g's stage-2 PE work is
            # emitted S2_DEFER chunks into group g+1 (its DVE copy is emitted
            # immediately and runs under these matmuls)
            pending_s2 = None
            for gi, ng in enumerate(GROUPS):
                a_s = a_ss[gi]
                for k in range(NCH):
                    if k == S2_DEFER and pending_s2 is not None:
                        pending_s2()
                        pending_s2 = None
                    nc.tensor.matmul(
                        aggps[gi][:],
                        lhsT=f_s[:, k, :],
                        rhs=a_s[:, k, :],
                        start=(k == 0),
                        stop=(k == NCH - 1),
                    )
                pending_s2 = make_stage2(gi, GROUP_WBASE[gi])
            pending_s2()

    nc.compile()
    return nc


def _prep_inputs(features, edge_index, W, b):
    """Host-side sharding: dense per-core fp8 count matrices + packed F/W/b."""
    src = np.asarray(edge_index[0]).astype(np.int64)
    dst = np.asarray(edge_index[1]).astype(np.int64)

    # A[core, p, c, dloc] = #edges (src = c*128+p) -> (dst = core*1280+dloc)
    A = np.zeros((N_CORES, P, NCH, DPC), np.uint8)
    flat = ((dst // DPC * P + src % P) * NCH + src // P) * DPC + dst % DPC
    np.add.at(A.reshape(-1), flat, 1)
    amax = int(A.max())
    assert amax <= 16, f"edge multiplicity {amax} not exact in fp8"
    A8 = A.astype(ml_dtypes.float8_e4m3)

    a_groups = [
        np.ascontiguousarray(A8[:, :, :, lo:hi]) for lo, hi in GROUP_BOUNDS
    ]

    f16 = np.zeros((NCH * P, C_IN), ml_dtypes.bfloat16)
    f16[:N_NODES] = np.asarray(features, np.float32).astype(ml_dtypes.bfloat16)
    f_host = np.ascontiguousarray(f16.reshape(NCH, P, C_IN).transpose(1, 0, 2))
    w_host = np.asarray(W, np.float32).astype(ml_dtypes.bfloat16)
    bb_host = np.tile(np.asarray(b, np.float32)[None, :], (P, 1))

    in_maps = []
    for ci in range(N_CORES):
        m = {f"a{gi}": a_groups[gi][ci] for gi in range(len(GROUPS))}
        m.update({"f": f_host, "w": w_host, "bb": bb_host})
        in_maps.append(m)
    return in_maps


_KERNEL_CACHE: dict = {}


def _get_kernel():
    if "k" not in _KERNEL_CACHE:
        _KERNEL_CACHE["k"] = _build_kernel()
    return _KERNEL_CACHE["k"]


def kernel(features, edge_index, W, b):
    features = np.asarray(features, dtype=np.float32)
    edge_index = np.asarray(edge_index)
    W = np.asarray(W, dtype=np.float32)
    b = np.asarray(b, dtype=np.float32)
    assert features.shape == (N_NODES, C_IN), features.shape
    assert W.shape == (C_IN, C_OUT) and b.shape == (C_OUT,)

    in_maps = _prep_inputs(features, edge_index, W, b)
    nc = _get_kernel()
    res = bass_utils.run_bass_kernel_spmd(nc, in_maps, core_ids=list(range(N_CORES)))
    # out is [128, 10, 256] f16 p-major per core -> first 1250 rows valid
    out = np.concatenate(
        [
            np.asarray(res.results[c]["out"], np.float32)
            .transpose(1, 0, 2)
            .reshape(NWIN * P, C_OUT)[:DPC]
            for c in range(N_CORES)
        ],
        axis=0,
    )
    return np.ascontiguousarray(out)


# revision 21
# speedup vs baseline: 1.0511x; 1.0511x over previous
"""nn_GCNConv Trainium2 Bass kernel (8 NeuronCores, SPMD, no collectives).

Computation: out = segment_sum(features[src], dst, N) @ W + b
  features [10000,128] f32, edge_index [2,640000] i64, W [128,256], b [256]

Strategy (dense count-matrix SpMM -> pure streaming GEMM, no SWDGE gather):
  - The segment-sum is  agg = A @ F  where A[d,s] = #edges s->d.  The host
    builds A as a dense fp8 count matrix (counts are tiny ints, exact in
    fp8e4) sharded by dst: core c owns dst nodes [1280c, 1280c+1280).
  - dst is sharded 1250/core (no dst padding); GROUP-MAJOR schedule:
    F ([128,79,128] bf16) lives fully in SBUF; per dst group g
    (512/512/226 columns) the PE accumulates
      aggT[f,dg] += F_chunk[s,f]^T @ A_g_chunk[s,dg]
    over all 79 src chunks into one persistent PSUM bank, then group g's
    stage 2 (per-window PSUM->SBUF copy, aggT^T @ W + b, output DMA) runs
    while group g+1's A is still streaming -- only the last (smallest)
    group's stage 2 is on the tail.  Stage-2 PE matmuls are emitted a few
    chunks into the next group so the DVE copies hide under accumulation
    matmuls (and fill the early wire-starved phase).
  - A streams on the sync HWDGE ring in consumption order (g0,g1,g2); F
    streams on the scalar ring ahead of its per-chunk use, followed by
    w/b and the output stores (keeping the early SDMA round-robin 2-way).
    Slabs ramp small->~0.9MB: the 8 shared DMA-lane sems cap DMAs in
    flight, so tiny slabs choke the ~420 GB/s wire, while coarse slabs
    stall the PE at slab-sem boundaries (+~2us completion receipt each).
    Zero-weight warm-up matmuls bridge the DMA head so the PE HAM clock
    gate reaches 2.4 GHz before real work and never re-throttles.
  - walrus's codegen epilogue clears all 256 semaphores one
    EVENT_SEMAPHORE each (~6.6us, unavoidable); max-sem-num=78 keeps bass
    sems compact (harmless, kept from the attempt to shrink that loop).
  - Output is written p-major ([128,10,256] f16, last window 98 rows
    valid) so stores are big-line DMAs; the host untransposes.  Host does
    only index bookkeeping (histogram/pack); all feature arithmetic runs
    on device.
"""

import sys

import ml_dtypes
import numpy as np

_TRN_REPO = "/opt/trn_rl_repo"
if _TRN_REPO not in sys.path:
    sys.path.insert(0, _TRN_REPO)

import concourse.bass as bass  # noqa: E402
import concourse.mybir as mybir  # noqa: E402
import concourse.tile as tile  # noqa: E402
from concourse import bacc, bass_utils  # noqa: E402

# ---------------------------------------------------------------------------
# The walrus codegen epilogue clears every semaphore in the 256-entry file
# one EVENT_SEMAPHORE per sem (~115 ns each, split across 5 engines) — ~6 us
# of measured kernel time.  The sem file split is governed by max-sem-num
# (walrus owns [0, N), bass kernels own [N, 256)); shrinking N and telling
# bass to allocate in the small range right above it shrinks the clear loop.
# 78 is the documented-safe walrus minimum (the RDH configuration value).
_MAX_SEM_NUM = 78


def _patched_max_sem_num() -> int:
    return _MAX_SEM_NUM


bass.get_walrus_max_sem_num = _patched_max_sem_num

_orig_bir_verify = bass_utils.bir_verify_and_optimise


def _bir_verify_with_sem_cap(*args, **kwargs):
    orig_get_args = bass_utils.get_walrus_args

    def patched_get_args(*a, **k):
        return orig_get_args(*a, **k) + [f"--max-sem-num={_MAX_SEM_NUM}"]

    bass_utils.get_walrus_args = patched_get_args
    try:
        return _orig_bir_verify(*args, **kwargs)
    finally:
        bass_utils.get_walrus_args = orig_get_args


bass_utils.bir_verify_and_optimise = _bir_verify_with_sem_cap
# ---------------------------------------------------------------------------

# ---------------------------------------------------------------------------
# Workaround: this walrus build rejects >1 sync-wait on a CTRL instruction
# ("Too many sync wait commands"). Tile's tail drain attaches a wait for every
# live sem lane to one InstDrain; chunk them across single-wait nops instead.
import re as _re  # noqa: E402

import bass_rust as _bass_rust  # noqa: E402


def _clock_ticks(vc):
    m = _re.search(r"\[([0-9, ]*)\]", repr(vc))
    return [int(x) for x in m.group(1).split(",")] if m.group(1).strip() else []


def _drain_and_barrier(self, tick_clock, wait_clock):
    ticks = _clock_ticks(tick_clock.global_clock)
    nz = [(i, t) for i, t in enumerate(ticks) if t > 0]
    for i, t in nz:
        vc = _bass_rust.VectorClock()
        vc.require_at_least(i, t)
        nop = self.nc.sync.nop(nofuse=True, hint="tail_wait")
        wait_clock.add_sem_waits(nop.ins, tile.ScopedClock({None: vc}))
    self.nc.sync.drain()  # waits already carried by the nops (SP FIFO order)
    self.nc.all_engine_barrier()
    assert self.sems is not None
    popped = self.nc._tile_sem_poison_stack.pop()
    assert popped is self._sem_poison
    self.nc.clear_and_free_semaphores(list(self.sems.allocated().values()))
    self.nc.all_engine_barrier()


tile.TileContext._drain_and_barrier = _drain_and_barrier
# ---------------------------------------------------------------------------

P = 128            # SBUF partitions = window node count = src chunk size
C_IN = 128
C_OUT = 256
N_NODES = 10000
N_CORES = 8
DPC = 1250         # dst nodes per core (exactly 10000/8 -- no dst padding)
NWIN = 10          # output windows per core (last window only 98 rows valid)
NCH = 79           # src chunks (10112 padded src rows / 128)
# Group order 512,512,226: the wide groups keep the PE stream dense early
# (a small-group-first variant starved the PE into HAM re-throttling);
# the small group last also gives the shortest stage-2 tail.
GROUPS = (512, 512, 226)   # dst columns per PSUM accumulation group
GROUP_BOUNDS = ((0, 512), (512, 1024), (1024, 1250))
GROUP_WBASE = (0, 4, 8)
# Slab sizes (src chunks per DMA).  All A slabs ride the sync HWDGE ring in
# consumption order (g0, g1, g2); F rides the scalar ring ahead of its use,
# followed by the output stores.  Slabs are kept >=0.6 MB mid-stream — the
# 8 shared DMA-lane sems cap DMAs in flight, and with small slabs the
# in-flight byte pool can't sustain the ~420 GB/s the wire delivers.
A_SLABS = (
    (4, 6, 8, 10, 12, 13, 13, 13),
    (13, 13, 13, 13, 13, 14),
    (20, 20, 20, 19),
)
F_SLABS = (4, 6, 8, 10, 12, 13, 13, 13)
N_WARM = 14        # dummy matmuls bridging the DMA head (HAM pre-warm)
S2_DEFER = 6      # emit group g's stage-2 this many chunks into group g+1


def _build_kernel():
    nc = bacc.Bacc("TRN2")
    dt = mybir.dt

    a_d = [
        nc.dram_tensor(f"a{gi}", [P, NCH, ng], dt.float8e4, kind="ExternalInput")
        for gi, ng in enumerate(GROUPS)
    ]
    f_d = nc.dram_tensor("f", [P, NCH, C_IN], dt.bfloat16, kind="ExternalInput")
    w_d = nc.dram_tensor("w", [C_IN, C_OUT], dt.bfloat16, kind="ExternalInput")
    bb_d = nc.dram_tensor("bb", [P, C_OUT], dt.float32, kind="ExternalInput")
    out_d = nc.dram_tensor("out", [P, NWIN, C_OUT], dt.float16, kind="ExternalOutput")

    with tile.TileContext(nc) as tc:
        with (
            tc.tile_pool(name="consts", bufs=1) as cpool,
            tc.tile_pool(name="a", bufs=1) as apool,
            tc.tile_pool(name="agg", bufs=1) as aggpool,
            tc.tile_pool(name="o", bufs=2) as opool,
            tc.tile_pool(name="psa", bufs=1, space="PSUM") as psa,
            tc.tile_pool(name="pso", bufs=1, space="PSUM") as pso,
        ):
            # HAM pre-warm: dummy matmuls on zeroed scratch keep the PE busy
            # during the DMA head so the real stream hits 2.4 GHz sooner.
            warm_w = cpool.tile([P, C_IN], dt.bfloat16)
            warm_x = cpool.tile([P, 256], dt.float8e4)
            warm_p = psa.tile([P, 256], dt.float32, tag="warm", bufs=1)
            nc.gpsimd.memset(warm_w[:], 0.0)
            nc.gpsimd.memset(warm_x[:], 0.0)
            for _ in range(N_WARM):
                nc.tensor.matmul(warm_p[:], lhsT=warm_w[:], rhs=warm_x[:],
                                 start=True, stop=True)

            w_s = cpool.tile([C_IN, C_OUT], dt.bfloat16)
            bb_s = cpool.tile([P, C_OUT], dt.float32)

            # F fully resident + per-group A tiles.  All input slabs are
            # striped across the sync/scalar HWDGE rings in consumption
            # order (region tracking gates each chunk's MM on the slab that
            # carries it).
            f_s = cpool.tile([P, NCH, C_IN], dt.bfloat16)
            a_ss = [
                apool.tile([P, NCH, ng], dt.float8e4, tag=f"g{gi}",
                           name=f"a_s{gi}")
                for gi, ng in enumerate(GROUPS)
            ]

            def slab(ring, dst, src, kb0, kbn):
                ring.dma_start(
                    out=dst[:, kb0 : kb0 + kbn, :], in_=src[:, kb0 : kb0 + kbn, :]
                )

            kb0 = 0
            for kbn in F_SLABS:
                slab(nc.scalar, f_s, f_d, kb0, kbn)
                kb0 += kbn
            # w/b ride the scalar ring AFTER F: issuing them early would make
            # the SDMA round-robin 3-way during the critical F+g0 phase
            nc.scalar.dma_start(out=w_s[:], in_=w_d[:])
            nc.scalar.dma_start(out=bb_s[:], in_=bb_d[:])
            for gi, slabs in enumerate(A_SLABS):
                kb0 = 0
                for kbn in slabs:
                    slab(nc.sync, a_ss[gi], a_d[gi], kb0, kbn)
                    kb0 += kbn

            # persistent PSUM accumulator per group (one bank each)
            aggps = [
                psa.tile([P, ng], dt.float32, tag=f"aggp{gi}", name=f"aggp{gi}")
                for gi, ng in enumerate(GROUPS)
            ]

            def make_stage2(gi, wbase):
                ng = GROUPS[gi]
                nw = (ng + P - 1) // P
                agg_s = aggpool.tile([P, ng], dt.bfloat16, tag=f"agg{gi}",
                                     name=f"agg_s{gi}")
                # per-window copies: window w's stage-2 matmul can start as
                # soon as its slice is copied (shorter tail on the last group)
                for wl in range(nw):
                    w0, w1 = wl * P, min((wl + 1) * P, ng)
                    nc.vector.tensor_copy(agg_s[:, w0:w1], aggps[gi][:, w0:w1])

                def emit_pe():
                    outp = pso.tile([P, nw, C_OUT], dt.float32, tag="op",
                                    padded_shape=[P, 4, C_OUT], name=f"op{gi}")
                    for wl in range(nw):
                        w0, w1 = wl * P, min((wl + 1) * P, ng)
                        nc.tensor.matmul(
                            outp[0 : w1 - w0, wl, :],
                            lhsT=agg_s[:, w0:w1],
                            rhs=w_s[:],
                            start=True,
                            stop=True,
                        )
                    out_t = opool.tile([P, nw, C_OUT], dt.float16, tag="o",
                                       padded_shape=[P, 4, C_OUT], name=f"out_g{gi}")
                    for wl in range(nw):
                        w0, w1 = wl * P, min((wl + 1) * P, ng)
                        nc.vector.tensor_add(out_t[0 : w1 - w0, wl, :],
                                             outp[0 : w1 - w0, wl, :],
                                             bb_s[0 : w1 - w0, :])
                    nc.scalar.dma_start(
                        out=out_d[:, wbase : wbase + nw, :], in_=out_t[:]
                    )

                return emit_pe

            # stage 1 accumulation, group-major; group g's stage-2 PE work is
            # emitted S2_DEFER chunks into group g+1 (its DVE copy is emitted
            # immediately and runs under these matmuls)
            pending_s2 = None
            for gi, ng in enumerate(GROUPS):
                a_s = a_ss[gi]
                for k in range(NCH):
                    if k == S2_DEFER and pending_s2 is not None:
                        pending_s2()
                        pending_s2 = None
                    nc.tensor.matmul(
                        aggps[gi][:],
                        lhsT=f_s[:, k, :],
                        rhs=a_s[:, k, :],
                        start=(k == 0),
                        stop=(k == NCH - 1),
                    )
                pending_s2 = make_stage2(gi, GROUP_WBASE[gi])
            pending_s2()

    nc.compile()
    return nc


def _prep_inputs(features, edge_index, W, b):
    """Host-side sharding: dense per-core fp8 count matrices + packed F/W/b."""
    src = np.asarray(edge_index[0]).astype(np.int64)
    dst = np.asarray(edge_index[1]).astype(np.int64)

    # A[core, p, c, dloc] = #edges (src = c*128+p) -> (dst = core*1280+dloc)
    A = np.zeros((N_CORES, P, NCH, DPC), np.uint8)
    flat = ((dst // DPC * P + src % P) * NCH + src // P) * DPC + dst % DPC
    np.add.at(A.reshape(-1), flat, 1)
    amax = int(A.max())
    assert amax <= 16, f"edge multiplicity {amax} not exact in fp8"
    A8 = A.astype(ml_dtypes.float8_e4m3)

    a_groups = [
        np.ascontiguousarray(A8[:, :, :, lo:hi]) for lo, hi in GROUP_BOUNDS
    ]

    f16 = np.zeros((NCH * P, C_IN), ml_dtypes.bfloat16)
    f16[:N_NODES] = np.asarray(features, np.float32).astype(ml_dtypes.bfloat16)
    f_host = np.ascontiguousarray(f16.reshape(NCH, P, C_IN).transpose(1, 0, 2))
    w_host = np.asarray(W, np.float32).astype(ml_dtypes.bfloat16)
    bb_host = np.tile(np.asarray(b, np.float32)[None, :], (P, 1))

    in_maps = []
    for ci in range(N_CORES):
        m = {f"a{gi}": a_groups[gi][ci] for gi in range(len(GROUPS))}
        m.update({"f": f_host, "w": w_host, "bb": bb_host})
        in_maps.append(m)
    return in_maps


_KERNEL_CACHE: dict = {}


def _get_kernel():
    if "k" not in _KERNEL_CACHE:
        _KERNEL_CACHE["k"] = _build_kernel()
    return _KERNEL_CACHE["k"]


def kernel(features, edge_index, W, b):
    features = np.asarray(features, dtype=np.float32)
    edge_index = np.asarray(edge_index)
    W = np.asarray(W, dtype=np.float32)
    b = np.asarray(b, dtype=np.float32)
    assert features.shape == (N_NODES, C_IN), features.shape
    assert W.shape == (C_IN, C_OUT) and b.shape == (C_OUT,)

    in_maps = _prep_inputs(features, edge_index, W, b)
    nc = _get_kernel()
    res = bass_utils.run_bass_kernel_spmd(nc, in_maps, core_ids=list(range(N_CORES)))
    # out is [128, 10, 256] f16 p-major per core -> first 1250 rows valid
    out = np.concatenate(
        [
            np.asarray(res.results[c]["out"], np.float32)
            .transpose(1, 0, 2)
            .reshape(NWIN * P, C_OUT)[:DPC]
            for c in range(N_CORES)
        ],
        axis=0,
    )
    return np.ascontiguousarray(out)
